# revision 24
# baseline (speedup 1.0000x reference)
"""Chamfer loss (nn_ChamferLoss) on 8 Trainium2 NeuronCores.

V3: rank-window pruned brute force.

Host sorts both clouds by x. Core c owns the 1024-target slab of sorted rank
[1024c, 1024c+1024) and scans it against the W=2048 output points nearest in
sorted rank (a window centered on the slab, clipped at the ends). For sorted
gaussian clouds the true nearest neighbour lies inside that window for all but
a handful of points; every point carries a certificate (row-min <= squared
x-gap to the uncovered region) checked on the host, and uncertified points are
recomputed exactly on the host (a few points, exact patch).

Distance tiles are computed on the PE as K=24 bf16 matmuls (3-limb bf16
decomposition of fp32 -> fp32-accurate d2 at full bf16 PE rate).  Per
[128,2048] PSUM tile: ACT evacuates to a bf16 SBUF copy; DVE folds the copy
for the row-min (2x-packed bf16 tensor_tensor min) and accumulates the
column-min across the 8 blocks.  dist1 row-mins and the per-core column-min
window go back to the host, which folds partitions/cores, applies the
certificates, patches, and finishes sqrt/mean/scale.
"""

import sys

sys.path.insert(0, "/opt/trn_rl_repo")

import numpy as np
import ml_dtypes

N = 8192           # points per cloud
NCORES = 8
NPC = N // NCORES  # 1024 targets per core
P = 128
BLKS = NPC // P    # 8 blocks per core
K = 24             # contraction rows (3-limb decomposition)
CH = 512           # matmul free dim (one PSUM bank fp32)
W = 1024           # output-point window per core
CERT_MARGIN = 1.01 # bf16 slack when checking certificates

_BUILT = None


def _limbs(x):
    h = x.astype(ml_dtypes.bfloat16).astype(np.float32)
    r = x - h
    m = r.astype(ml_dtypes.bfloat16).astype(np.float32)
    l = (r - m).astype(ml_dtypes.bfloat16).astype(np.float32)
    return h, m, l


def _stationary_rows(pts):
    """[24, n] lhsT rows: coord limbs + |p|^2 limbs + ones."""
    ph, pm, pl = _limbs(pts)
    p2 = np.sum(pts.astype(np.float64) ** 2, -1).astype(np.float32)
    p2h, p2m, p2l = _limbs(p2)
    one = np.ones_like(p2)
    return np.stack(
        [ph[:, 0], ph[:, 1], ph[:, 2],
         ph[:, 0], ph[:, 1], ph[:, 2],
         pm[:, 0], pm[:, 1], pm[:, 2],
         ph[:, 0], ph[:, 1], ph[:, 2],
         pl[:, 0], pl[:, 1], pl[:, 2],
         pm[:, 0], pm[:, 1], pm[:, 2],
         p2h, p2m, p2l,
         one, one, one], 0)


def _moving_rows(pts):
    """[24, n] rhs rows, limb-paired with _stationary_rows."""
    qh, qm, ql = _limbs(pts)
    q2 = np.sum(pts.astype(np.float64) ** 2, -1).astype(np.float32)
    q2h, q2m, q2l = _limbs(q2)
    one = np.ones_like(q2)
    return np.stack(
        [-2 * qh[:, 0], -2 * qh[:, 1], -2 * qh[:, 2],
         -2 * qm[:, 0], -2 * qm[:, 1], -2 * qm[:, 2],
         -2 * qh[:, 0], -2 * qh[:, 1], -2 * qh[:, 2],
         -2 * ql[:, 0], -2 * ql[:, 1], -2 * ql[:, 2],
         -2 * qh[:, 0], -2 * qh[:, 1], -2 * qh[:, 2],
         -2 * qm[:, 0], -2 * qm[:, 1], -2 * qm[:, 2],
         one, one, one,
         q2h, q2m, q2l], 0)


def _build():
    global _BUILT
    if _BUILT is not None:
        return _BUILT

    import concourse.bacc as bacc
    import concourse.mybir as mybir
    import concourse.tile as tile

    f32 = mybir.dt.float32
    bf16 = mybir.dt.bfloat16
    MIN = mybir.AluOpType.min
    X = mybir.AxisListType.X

    nc = bacc.Bacc(None, target_bir_lowering=False, debug=False)
    wts = nc.declare_dram_parameter("wts", [K, NPC], bf16, isOutput=False)
    rhs = nc.declare_dram_parameter("rhs", [K, W], bf16, isOutput=False)
    rowout_d = nc.declare_dram_parameter("rowout", [P, BLKS * (W // 4)], bf16, isOutput=True)
    colout_d = nc.declare_dram_parameter("colout", [P, W], bf16, isOutput=True)

    with tile.TileContext(nc) as tc:
        with tc.tile_pool(name="const", bufs=1) as cpool, \
             tc.tile_pool(name="cp", bufs=8) as cppool, \
             tc.tile_pool(name="ps", bufs=2, space="PSUM") as pspool:
            w_t = cpool.tile([K, NPC], bf16, name="w_t")
            r_t = cpool.tile([K, W], bf16, name="r_t")
            # chunked inputs so block 0's matmuls start early
            nc.sync.dma_start(out=w_t[:, :], in_=wts[:, :])
            for j in range(W // CH):
                nc.sync.dma_start(
                    out=r_t[:, j * CH:(j + 1) * CH], in_=rhs[:, j * CH:(j + 1) * CH])

            colacc = cpool.tile([P, W], bf16, name="colacc")
            s1 = cpool.tile([P, W // 2], bf16, name="s1")
            rowfold = cpool.tile([P, BLKS * (W // 4)], bf16, name="rowfold")

            for b in range(BLKS):
                lhsT = w_t[:, b * P:(b + 1) * P]
                pst = pspool.tile([P, W], f32, name="pst")
                for j in range(W // CH):
                    nc.tensor.matmul(
                        out=pst[:, j * CH:(j + 1) * CH],
                        lhsT=lhsT,
                        rhs=r_t[:, j * CH:(j + 1) * CH],
                    )
                if b == 0:
                    cp_t = colacc                 # block 0 copy IS the init
                else:
                    cp_t = cppool.tile([P, W], bf16, name="cp", tag="cp")
                nc.scalar.copy(out=cp_t[:, :], in_=pst[:, :])

                # column accumulation first (feeds the final colout DMA)
                if b > 0:
                    nc.vector.tensor_tensor(
                        out=colacc, in0=cp_t, in1=colacc, op=MIN)

                # row-min: fold the copy W -> W/2 -> W/4; host finishes the min
                nc.vector.tensor_tensor(
                    out=s1, in0=cp_t[:, 0:W // 2], in1=cp_t[:, W // 2:W], op=MIN)
                fb = rowfold[:, b * (W // 4):(b + 1) * (W // 4)]
                nc.vector.tensor_tensor(
                    out=fb, in0=s1[:, 0:W // 4], in1=s1[:, W // 4:W // 2], op=MIN)
                nc.scalar.dma_start(
                    out=rowout_d[:, b * (W // 4):(b + 1) * (W // 4)], in_=fb)

            nc.sync.dma_start(out=colout_d[:, :], in_=colacc[:, :])

    nc.compile()
    _BUILT = nc
    return nc


def _window(c):
    center = c * NPC + NPC // 2
    lo = min(max(0, center - W // 2), N - W)
    return lo, lo + W


def kernel(target, output, cur, substeps):
    from concourse.bass_utils import run_bass_kernel_spmd

    a = np.asarray(target, dtype=np.float32)[0]   # (8192,3) target cloud
    b = np.asarray(output, dtype=np.float32)[0]   # (8192,3) output cloud
    cur = int(np.asarray(cur))
    substeps = int(np.asarray(substeps))

    sa = np.argsort(a[:, 0], kind="stable")
    sb = np.argsort(b[:, 0], kind="stable")
    A = a[sa]                                     # sorted targets
    B = b[sb]                                     # sorted outputs

    bf = ml_dtypes.bfloat16
    w_full = _stationary_rows(A).astype(bf)       # [24, 8192]
    r_full = _moving_rows(B).astype(bf)           # [24, 8192]

    in_maps = []
    for c in range(NCORES):
        lo, hi = _window(c)
        in_maps.append({
            "wts": np.ascontiguousarray(w_full[:, c * NPC:(c + 1) * NPC]),
            "rhs": np.ascontiguousarray(r_full[:, lo:hi]),
        })

    nc = _build()
    results = run_bass_kernel_spmd(nc, in_maps, list(range(NCORES))).results

    A64 = A.astype(np.float64)
    B64 = B.astype(np.float64)
    a2 = np.sum(A64 ** 2, 1)
    b2 = np.sum(B64 ** 2, 1)

    # ---- dist1 (per sorted target) ----
    d1 = np.empty(N, np.float64)
    col_parts = []
    for c in range(NCORES):
        rf = results[c]["rowout"].astype(np.float32)      # [128, BLKS*(W//4)]
        rmins = rf.reshape(P, BLKS, W // 4).min(axis=2)   # [128, BLKS]
        d1[c * NPC:(c + 1) * NPC] = rmins.T.reshape(-1)
        col_parts.append(results[c]["colout"].astype(np.float32).min(axis=0))

    # dist1 certificates: squared x-gap to the uncovered ranks
    bad1 = []
    for c in range(NCORES):
        lo, hi = _window(c)
        t = slice(c * NPC, (c + 1) * NPC)
        gl = (A[t, 0] - B[lo - 1, 0]) ** 2 if lo > 0 else np.full(NPC, np.inf)
        gr = (B[hi, 0] - A[t, 0]) ** 2 if hi < N else np.full(NPC, np.inf)
        fail = d1[t] * CERT_MARGIN > np.minimum(gl, gr)
        bad1.extend((c * NPC + np.nonzero(fail)[0]).tolist())
    for t in bad1:
        d1[t] = np.min(a2[t] + b2 - 2.0 * (B64 @ A64[t]))

    # ---- dist2 (per sorted output) ----
    d2 = np.full(N, np.inf, np.float64)
    cov_lo = np.full(N, N, np.int64)
    cov_hi = np.zeros(N, np.int64)
    for c in range(NCORES):
        lo, hi = _window(c)
        np.minimum.at(d2, np.arange(lo, hi), col_parts[c].astype(np.float64))
        cov_lo[lo:hi] = np.minimum(cov_lo[lo:hi], c * NPC)
        cov_hi[lo:hi] = np.maximum(cov_hi[lo:hi], (c + 1) * NPC)
    gl = np.where(cov_lo > 0, (B[:, 0] - A[np.maximum(cov_lo - 1, 0), 0]) ** 2, np.inf)
    gr = np.where(cov_hi < N, (A[np.minimum(cov_hi, N - 1), 0] - B[:, 0]) ** 2, np.inf)
    bad2 = np.nonzero(d2 * CERT_MARGIN > np.minimum(gl, gr))[0]
    for j in bad2:
        d2[j] = np.min(b2[j] + a2 - 2.0 * (A64 @ B64[j]))

    m1 = np.sqrt(np.maximum(d1, 0.0)).mean()
    m2 = np.sqrt(np.maximum(d2, 0.0)).mean()
    loss = 0.5 * (m1 + m2)
    scale = 10.0 / (0.99 ** (cur // substeps))
    return np.float32(loss * scale)


# revision 25
# speedup vs baseline: 1.0496x; 1.0496x over previous
"""Chamfer loss (nn_ChamferLoss) on 8 Trainium2 NeuronCores.

V3: rank-window pruned brute force.

Host sorts both clouds by x. Core c owns the 1024-target slab of sorted rank
[1024c, 1024c+1024) and scans it against the W=2048 output points nearest in
sorted rank (a window centered on the slab, clipped at the ends). For sorted
gaussian clouds the true nearest neighbour lies inside that window for all but
a handful of points; every point carries a certificate (row-min <= squared
x-gap to the uncovered region) checked on the host, and uncertified points are
recomputed exactly on the host (a few points, exact patch).

Distance tiles are computed on the PE as K=24 bf16 matmuls (3-limb bf16
decomposition of fp32 -> fp32-accurate d2 at full bf16 PE rate).  Per
[128,2048] PSUM tile: ACT evacuates to a bf16 SBUF copy; DVE folds the copy
for the row-min (2x-packed bf16 tensor_tensor min) and accumulates the
column-min across the 8 blocks.  dist1 row-mins and the per-core column-min
window go back to the host, which folds partitions/cores, applies the
certificates, patches, and finishes sqrt/mean/scale.
"""

import sys

sys.path.insert(0, "/opt/trn_rl_repo")

import numpy as np
import ml_dtypes

N = 8192           # points per cloud
NCORES = 8
NPC = N // NCORES  # 1024 targets per core
P = 128
BLKS = NPC // P    # 8 blocks per core
K = 24             # contraction rows (3-limb decomposition)
CH = 512           # matmul free dim (one PSUM bank fp32)
W = 1024           # output-point window per core
CERT_MARGIN = 1.01 # bf16 slack when checking certificates

_BUILT = None


def _limbs(x):
    h = x.astype(ml_dtypes.bfloat16).astype(np.float32)
    r = x - h
    m = r.astype(ml_dtypes.bfloat16).astype(np.float32)
    l = (r - m).astype(ml_dtypes.bfloat16).astype(np.float32)
    return h, m, l


def _stationary_rows(pts):
    """[24, n] lhsT rows: coord limbs + |p|^2 limbs + ones."""
    ph, pm, pl = _limbs(pts)
    p2 = np.sum(pts.astype(np.float64) ** 2, -1).astype(np.float32)
    p2h, p2m, p2l = _limbs(p2)
    one = np.ones_like(p2)
    return np.stack(
        [ph[:, 0], ph[:, 1], ph[:, 2],
         ph[:, 0], ph[:, 1], ph[:, 2],
         pm[:, 0], pm[:, 1], pm[:, 2],
         ph[:, 0], ph[:, 1], ph[:, 2],
         pl[:, 0], pl[:, 1], pl[:, 2],
         pm[:, 0], pm[:, 1], pm[:, 2],
         p2h, p2m, p2l,
         one, one, one], 0)


def _moving_rows(pts):
    """[24, n] rhs rows, limb-paired with _stationary_rows."""
    qh, qm, ql = _limbs(pts)
    q2 = np.sum(pts.astype(np.float64) ** 2, -1).astype(np.float32)
    q2h, q2m, q2l = _limbs(q2)
    one = np.ones_like(q2)
    return np.stack(
        [-2 * qh[:, 0], -2 * qh[:, 1], -2 * qh[:, 2],
         -2 * qm[:, 0], -2 * qm[:, 1], -2 * qm[:, 2],
         -2 * qh[:, 0], -2 * qh[:, 1], -2 * qh[:, 2],
         -2 * ql[:, 0], -2 * ql[:, 1], -2 * ql[:, 2],
         -2 * qh[:, 0], -2 * qh[:, 1], -2 * qh[:, 2],
         -2 * qm[:, 0], -2 * qm[:, 1], -2 * qm[:, 2],
         one, one, one,
         q2h, q2m, q2l], 0)


def _build():
    global _BUILT
    if _BUILT is not None:
        return _BUILT

    import concourse.bacc as bacc
    import concourse.mybir as mybir
    import concourse.tile as tile

    f32 = mybir.dt.float32
    bf16 = mybir.dt.bfloat16
    MIN = mybir.AluOpType.min
    X = mybir.AxisListType.X

    nc = bacc.Bacc(None, target_bir_lowering=False, debug=False)
    wts = nc.declare_dram_parameter("wts", [K, NPC], bf16, isOutput=False)
    rhs = nc.declare_dram_parameter("rhs", [K, W], bf16, isOutput=False)
    rowout_d = nc.declare_dram_parameter("rowout", [P, BLKS * (W // 4)], bf16, isOutput=True)
    colout_d = nc.declare_dram_parameter("colout", [P, W], bf16, isOutput=True)

    with tile.TileContext(nc) as tc:
        with tc.tile_pool(name="const", bufs=1) as cpool, \
             tc.tile_pool(name="cp", bufs=8) as cppool, \
             tc.tile_pool(name="ps", bufs=2, space="PSUM") as pspool:
            w_t = cpool.tile([K, NPC], bf16, name="w_t")
            r_t = cpool.tile([K, W], bf16, name="r_t")
            # chunked inputs so block 0's matmuls start early
            nc.sync.dma_start(out=w_t[:, :], in_=wts[:, :])
            for j in range(W // CH):
                nc.sync.dma_start(
                    out=r_t[:, j * CH:(j + 1) * CH], in_=rhs[:, j * CH:(j + 1) * CH])

            colacc = cpool.tile([P, W], bf16, name="colacc")
            s1 = cpool.tile([P, W // 2], bf16, name="s1")
            rowfold = cpool.tile([P, BLKS * (W // 4)], bf16, name="rowfold")

            for b in range(BLKS):
                lhsT = w_t[:, b * P:(b + 1) * P]
                pst = pspool.tile([P, W], f32, name="pst")
                for j in range(W // CH):
                    nc.tensor.matmul(
                        out=pst[:, j * CH:(j + 1) * CH],
                        lhsT=lhsT,
                        rhs=r_t[:, j * CH:(j + 1) * CH],
                    )
                if b == 0:
                    cp_t = colacc                 # block 0 copy IS the init
                else:
                    cp_t = cppool.tile([P, W], bf16, name="cp", tag="cp")
                nc.scalar.copy(out=cp_t[:, :], in_=pst[:, :])

                # column accumulation first (feeds the final colout DMA)
                if b > 0:
                    nc.vector.tensor_tensor(
                        out=colacc, in0=cp_t, in1=colacc, op=MIN)

                # row-min: fold the copy W -> W/2 -> W/4; host finishes the min
                nc.vector.tensor_tensor(
                    out=s1, in0=cp_t[:, 0:W // 2], in1=cp_t[:, W // 2:W], op=MIN)
                fb = rowfold[:, b * (W // 4):(b + 1) * (W // 4)]
                nc.vector.tensor_tensor(
                    out=fb, in0=s1[:, 0:W // 4], in1=s1[:, W // 4:W // 2], op=MIN)
                nc.sync.dma_start(
                    out=rowout_d[:, b * (W // 4):(b + 1) * (W // 4)], in_=fb)

            nc.sync.dma_start(out=colout_d[:, :], in_=colacc[:, :])

    nc.compile()
    _BUILT = nc
    return nc


def _window(c):
    center = c * NPC + NPC // 2
    lo = min(max(0, center - W // 2), N - W)
    return lo, lo + W


def kernel(target, output, cur, substeps):
    from concourse.bass_utils import run_bass_kernel_spmd

    a = np.asarray(target, dtype=np.float32)[0]   # (8192,3) target cloud
    b = np.asarray(output, dtype=np.float32)[0]   # (8192,3) output cloud
    cur = int(np.asarray(cur))
    substeps = int(np.asarray(substeps))

    sa = np.argsort(a[:, 0], kind="stable")
    sb = np.argsort(b[:, 0], kind="stable")
    A = a[sa]                                     # sorted targets
    B = b[sb]                                     # sorted outputs

    bf = ml_dtypes.bfloat16
    w_full = _stationary_rows(A).astype(bf)       # [24, 8192]
    r_full = _moving_rows(B).astype(bf)           # [24, 8192]

    in_maps = []
    for c in range(NCORES):
        lo, hi = _window(c)
        in_maps.append({
            "wts": np.ascontiguousarray(w_full[:, c * NPC:(c + 1) * NPC]),
            "rhs": np.ascontiguousarray(r_full[:, lo:hi]),
        })

    nc = _build()
    results = run_bass_kernel_spmd(nc, in_maps, list(range(NCORES))).results

    A64 = A.astype(np.float64)
    B64 = B.astype(np.float64)
    a2 = np.sum(A64 ** 2, 1)
    b2 = np.sum(B64 ** 2, 1)

    # ---- dist1 (per sorted target) ----
    d1 = np.empty(N, np.float64)
    col_parts = []
    for c in range(NCORES):
        rf = results[c]["rowout"].astype(np.float32)      # [128, BLKS*(W//4)]
        rmins = rf.reshape(P, BLKS, W // 4).min(axis=2)   # [128, BLKS]
        d1[c * NPC:(c + 1) * NPC] = rmins.T.reshape(-1)
        col_parts.append(results[c]["colout"].astype(np.float32).min(axis=0))

    # dist1 certificates: squared x-gap to the uncovered ranks
    bad1 = []
    for c in range(NCORES):
        lo, hi = _window(c)
        t = slice(c * NPC, (c + 1) * NPC)
        gl = (A[t, 0] - B[lo - 1, 0]) ** 2 if lo > 0 else np.full(NPC, np.inf)
        gr = (B[hi, 0] - A[t, 0]) ** 2 if hi < N else np.full(NPC, np.inf)
        fail = d1[t] * CERT_MARGIN > np.minimum(gl, gr)
        bad1.extend((c * NPC + np.nonzero(fail)[0]).tolist())
    for t in bad1:
        d1[t] = np.min(a2[t] + b2 - 2.0 * (B64 @ A64[t]))

    # ---- dist2 (per sorted output) ----
    d2 = np.full(N, np.inf, np.float64)
    cov_lo = np.full(N, N, np.int64)
    cov_hi = np.zeros(N, np.int64)
    for c in range(NCORES):
        lo, hi = _window(c)
        np.minimum.at(d2, np.arange(lo, hi), col_parts[c].astype(np.float64))
        cov_lo[lo:hi] = np.minimum(cov_lo[lo:hi], c * NPC)
        cov_hi[lo:hi] = np.maximum(cov_hi[lo:hi], (c + 1) * NPC)
    gl = np.where(cov_lo > 0, (B[:, 0] - A[np.maximum(cov_lo - 1, 0), 0]) ** 2, np.inf)
    gr = np.where(cov_hi < N, (A[np.minimum(cov_hi, N - 1), 0] - B[:, 0]) ** 2, np.inf)
    bad2 = np.nonzero(d2 * CERT_MARGIN > np.minimum(gl, gr))[0]
    for j in bad2:
        d2[j] = np.min(b2[j] + a2 - 2.0 * (A64 @ B64[j]))

    m1 = np.sqrt(np.maximum(d1, 0.0)).mean()
    m2 = np.sqrt(np.maximum(d2, 0.0)).mean()
    loss = 0.5 * (m1 + m2)
    scale = 10.0 / (0.99 ** (cur // substeps))
    return np.float32(loss * scale)


# revision 26
# speedup vs baseline: 1.0841x; 1.0328x over previous
"""Chamfer loss (nn_ChamferLoss) on 8 Trainium2 NeuronCores.

V3: rank-window pruned brute force.

Host sorts both clouds by x. Core c owns the 1024-target slab of sorted rank
[1024c, 1024c+1024) and scans it against the W=2048 output points nearest in
sorted rank (a window centered on the slab, clipped at the ends). For sorted
gaussian clouds the true nearest neighbour lies inside that window for all but
a handful of points; every point carries a certificate (row-min <= squared
x-gap to the uncovered region) checked on the host, and uncertified points are
recomputed exactly on the host (a few points, exact patch).

Distance tiles are computed on the PE as K=24 bf16 matmuls (3-limb bf16
decomposition of fp32 -> fp32-accurate d2 at full bf16 PE rate).  Per
[128,2048] PSUM tile: ACT evacuates to a bf16 SBUF copy; DVE folds the copy
for the row-min (2x-packed bf16 tensor_tensor min) and accumulates the
column-min across the 8 blocks.  dist1 row-mins and the per-core column-min
window go back to the host, which folds partitions/cores, applies the
certificates, patches, and finishes sqrt/mean/scale.
"""

import sys

sys.path.insert(0, "/opt/trn_rl_repo")

import numpy as np
import ml_dtypes

N = 8192           # points per cloud
NCORES = 8
NPC = N // NCORES  # 1024 targets per core
P = 128
BLKS = NPC // P    # 8 blocks per core
K = 24             # contraction rows (3-limb decomposition)
CH = 512           # matmul free dim (one PSUM bank fp32)
W = 1024           # output-point window per core
CERT_MARGIN = 1.01 # bf16 slack when checking certificates

_BUILT = None


def _limbs(x):
    h = x.astype(ml_dtypes.bfloat16).astype(np.float32)
    r = x - h
    m = r.astype(ml_dtypes.bfloat16).astype(np.float32)
    l = (r - m).astype(ml_dtypes.bfloat16).astype(np.float32)
    return h, m, l


def _stationary_rows(pts):
    """[24, n] lhsT rows: coord limbs + |p|^2 limbs + ones."""
    ph, pm, pl = _limbs(pts)
    p2 = np.sum(pts.astype(np.float64) ** 2, -1).astype(np.float32)
    p2h, p2m, p2l = _limbs(p2)
    one = np.ones_like(p2)
    return np.stack(
        [ph[:, 0], ph[:, 1], ph[:, 2],
         ph[:, 0], ph[:, 1], ph[:, 2],
         pm[:, 0], pm[:, 1], pm[:, 2],
         ph[:, 0], ph[:, 1], ph[:, 2],
         pl[:, 0], pl[:, 1], pl[:, 2],
         pm[:, 0], pm[:, 1], pm[:, 2],
         p2h, p2m, p2l,
         one, one, one], 0)


def _moving_rows(pts):
    """[24, n] rhs rows, limb-paired with _stationary_rows."""
    qh, qm, ql = _limbs(pts)
    q2 = np.sum(pts.astype(np.float64) ** 2, -1).astype(np.float32)
    q2h, q2m, q2l = _limbs(q2)
    one = np.ones_like(q2)
    return np.stack(
        [-2 * qh[:, 0], -2 * qh[:, 1], -2 * qh[:, 2],
         -2 * qm[:, 0], -2 * qm[:, 1], -2 * qm[:, 2],
         -2 * qh[:, 0], -2 * qh[:, 1], -2 * qh[:, 2],
         -2 * ql[:, 0], -2 * ql[:, 1], -2 * ql[:, 2],
         -2 * qh[:, 0], -2 * qh[:, 1], -2 * qh[:, 2],
         -2 * qm[:, 0], -2 * qm[:, 1], -2 * qm[:, 2],
         one, one, one,
         q2h, q2m, q2l], 0)


def _build():
    global _BUILT
    if _BUILT is not None:
        return _BUILT

    import concourse.bacc as bacc
    import concourse.mybir as mybir
    import concourse.tile as tile

    f32 = mybir.dt.float32
    bf16 = mybir.dt.bfloat16
    MIN = mybir.AluOpType.min
    X = mybir.AxisListType.X

    nc = bacc.Bacc(None, target_bir_lowering=False, debug=False)
    wts = nc.declare_dram_parameter("wts", [K, NPC], bf16, isOutput=False)
    rhs = nc.declare_dram_parameter("rhs", [K, W], bf16, isOutput=False)
    rowout_d = nc.declare_dram_parameter("rowout", [P, BLKS * (W // 4)], bf16, isOutput=True)
    colout_d = nc.declare_dram_parameter("colout", [P, W], bf16, isOutput=True)

    with tile.TileContext(nc) as tc:
        with tc.tile_pool(name="const", bufs=1) as cpool, \
             tc.tile_pool(name="cp", bufs=8) as cppool, \
             tc.tile_pool(name="ps", bufs=2, space="PSUM") as pspool:
            w_t = cpool.tile([K, NPC], bf16, name="w_t")
            r_t = cpool.tile([K, W], bf16, name="r_t")
            # chunked inputs so block 0's matmuls start early
            nc.gpsimd.dma_start(out=w_t[:, :], in_=wts[:, :])
            for j in range(W // CH):
                nc.sync.dma_start(
                    out=r_t[:, j * CH:(j + 1) * CH], in_=rhs[:, j * CH:(j + 1) * CH])

            colacc = cpool.tile([P, W], bf16, name="colacc")
            s1 = cpool.tile([P, W // 2], bf16, name="s1")
            rowfold = cpool.tile([P, BLKS * (W // 4)], bf16, name="rowfold")

            for b in range(BLKS):
                lhsT = w_t[:, b * P:(b + 1) * P]
                pst = pspool.tile([P, W], f32, name="pst")
                for j in range(W // CH):
                    nc.tensor.matmul(
                        out=pst[:, j * CH:(j + 1) * CH],
                        lhsT=lhsT,
                        rhs=r_t[:, j * CH:(j + 1) * CH],
                    )
                if b == 0:
                    cp_t = colacc                 # block 0 copy IS the init
                else:
                    cp_t = cppool.tile([P, W], bf16, name="cp", tag="cp")
                nc.scalar.copy(out=cp_t[:, :], in_=pst[:, :])

                # column accumulation first (feeds the final colout DMA)
                if b > 0:
                    nc.vector.tensor_tensor(
                        out=colacc, in0=cp_t, in1=colacc, op=MIN)

                # row-min: fold the copy W -> W/2 -> W/4; host finishes the min
                nc.vector.tensor_tensor(
                    out=s1, in0=cp_t[:, 0:W // 2], in1=cp_t[:, W // 2:W], op=MIN)
                fb = rowfold[:, b * (W // 4):(b + 1) * (W // 4)]
                nc.vector.tensor_tensor(
                    out=fb, in0=s1[:, 0:W // 4], in1=s1[:, W // 4:W // 2], op=MIN)
                nc.sync.dma_start(
                    out=rowout_d[:, b * (W // 4):(b + 1) * (W // 4)], in_=fb)

            nc.sync.dma_start(out=colout_d[:, :], in_=colacc[:, :])

    nc.compile()
    _BUILT = nc
    return nc


def _window(c):
    center = c * NPC + NPC // 2
    lo = min(max(0, center - W // 2), N - W)
    return lo, lo + W


def kernel(target, output, cur, substeps):
    from concourse.bass_utils import run_bass_kernel_spmd

    a = np.asarray(target, dtype=np.float32)[0]   # (8192,3) target cloud
    b = np.asarray(output, dtype=np.float32)[0]   # (8192,3) output cloud
    cur = int(np.asarray(cur))
    substeps = int(np.asarray(substeps))

    sa = np.argsort(a[:, 0], kind="stable")
    sb = np.argsort(b[:, 0], kind="stable")
    A = a[sa]                                     # sorted targets
    B = b[sb]                                     # sorted outputs

    bf = ml_dtypes.bfloat16
    w_full = _stationary_rows(A).astype(bf)       # [24, 8192]
    r_full = _moving_rows(B).astype(bf)           # [24, 8192]

    in_maps = []
    for c in range(NCORES):
        lo, hi = _window(c)
        in_maps.append({
            "wts": np.ascontiguousarray(w_full[:, c * NPC:(c + 1) * NPC]),
            "rhs": np.ascontiguousarray(r_full[:, lo:hi]),
        })

    nc = _build()
    results = run_bass_kernel_spmd(nc, in_maps, list(range(NCORES))).results

    A64 = A.astype(np.float64)
    B64 = B.astype(np.float64)
    a2 = np.sum(A64 ** 2, 1)
    b2 = np.sum(B64 ** 2, 1)

    # ---- dist1 (per sorted target) ----
    d1 = np.empty(N, np.float64)
    col_parts = []
    for c in range(NCORES):
        rf = results[c]["rowout"].astype(np.float32)      # [128, BLKS*(W//4)]
        rmins = rf.reshape(P, BLKS, W // 4).min(axis=2)   # [128, BLKS]
        d1[c * NPC:(c + 1) * NPC] = rmins.T.reshape(-1)
        col_parts.append(results[c]["colout"].astype(np.float32).min(axis=0))

    # dist1 certificates: squared x-gap to the uncovered ranks
    bad1 = []
    for c in range(NCORES):
        lo, hi = _window(c)
        t = slice(c * NPC, (c + 1) * NPC)
        gl = (A[t, 0] - B[lo - 1, 0]) ** 2 if lo > 0 else np.full(NPC, np.inf)
        gr = (B[hi, 0] - A[t, 0]) ** 2 if hi < N else np.full(NPC, np.inf)
        fail = d1[t] * CERT_MARGIN > np.minimum(gl, gr)
        bad1.extend((c * NPC + np.nonzero(fail)[0]).tolist())
    for t in bad1:
        d1[t] = np.min(a2[t] + b2 - 2.0 * (B64 @ A64[t]))

    # ---- dist2 (per sorted output) ----
    d2 = np.full(N, np.inf, np.float64)
    cov_lo = np.full(N, N, np.int64)
    cov_hi = np.zeros(N, np.int64)
    for c in range(NCORES):
        lo, hi = _window(c)
        np.minimum.at(d2, np.arange(lo, hi), col_parts[c].astype(np.float64))
        cov_lo[lo:hi] = np.minimum(cov_lo[lo:hi], c * NPC)
        cov_hi[lo:hi] = np.maximum(cov_hi[lo:hi], (c + 1) * NPC)
    gl = np.where(cov_lo > 0, (B[:, 0] - A[np.maximum(cov_lo - 1, 0), 0]) ** 2, np.inf)
    gr = np.where(cov_hi < N, (A[np.minimum(cov_hi, N - 1), 0] - B[:, 0]) ** 2, np.inf)
    bad2 = np.nonzero(d2 * CERT_MARGIN > np.minimum(gl, gr))[0]
    for j in bad2:
        d2[j] = np.min(b2[j] + a2 - 2.0 * (A64 @ B64[j]))

    m1 = np.sqrt(np.maximum(d1, 0.0)).mean()
    m2 = np.sqrt(np.maximum(d2, 0.0)).mean()
    loss = 0.5 * (m1 + m2)
    scale = 10.0 / (0.99 ** (cur // substeps))
    return np.float32(loss * scale)
